# revision 1
# baseline (speedup 1.0000x reference)
"""Trainium2 Bass kernel for BertEmbedding segment-mean-pool + linear.

Reference computation (per batch element b):
    pooled[t, :] = mean_{s : word_ids[b,s]==t} hidden[b, s, :]   (0 if empty)
    pooled[t, :] = 0 where t >= token_lengths[b]
    out[b] = pooled @ W.T + b_bias                                [T, E]

Shapes: hidden [64, 512, 768] f32, word_ids [64, 512] i32 (sorted),
token_lengths [64] i32, W [512, 768] f32, b [512] f32 -> out [64, 256, 512].

Strategy: data-parallel over batch across 8 NeuronCores (8 sentences/core).
Per sentence, on device:
  1. one-hot oh[s, t] = (word_ids[s] == t) built via iota + is_equal (DVE)
  2. pooled_sums^T [h, t] = hidden^T-contraction via PE matmul
     (stationary = hidden tile [s, h-chunk], moving = one-hot [s, t]),
     counts [1, t] via ones-vector stationary.  fp32r (FP22) full-rate.
  3. scale[t] = (t < len) / max(counts, 1) in [1, T] row layout (len is a
     legal per-partition scalar there), transposed to per-partition scale
     columns [128, 1] via two tiny SBUF->SBUF DMAs
  4. pooled^T moved PSUM->SBUF by plain copies (split ACT/DVE)
  5. out[t-chunk] [128t, 512e] = sum_h pooled^T[h, tc]^T @ W^T[h, e]
  6. scale applied post-matmul as ACT per-partition scale (scale commutes
     through the linear), bias added via broadcast constant (DVE add),
     DMA out rows (2KB contiguous per partition)
  7. large DMAs split across both HWDGE rings (nc.sync + nc.scalar)
"""

import sys

if "/opt/trn_rl_repo" not in sys.path:
    sys.path.insert(0, "/opt/trn_rl_repo")

import numpy as np

B, S, H, E, T = 64, 512, 768, 512, 256
NCORES = 8
BL = B // NCORES  # sentences per core
KS = S // 128  # 4 s-tiles (contraction of matmul 1)
KH = H // 128  # 6 h-tiles (contraction of matmul 2)
CT = T // 128  # 2 t-chunks of the output

_cache: dict = {}


def _build(reps: int = 1, bufs: int = 2, psum_bufs: int = 1, dma2: int = 1, dma3: int = 0, widfirst: int = 0, outmerge: int = 0, dma4: int = 0, ptsdve: int = 0, ablate: str = ""):
    """Build + compile the per-core Bass program. Returns the Bacc object.

    reps > 1 repeats the whole per-core computation (used only for timing
    slope measurements in test.py). ablate: comma list of {mm1,mm2,cnt}
    for timing ablations (breaks correctness)."""
    ablated = set(ablate.split(",")) if ablate else set()
    from concourse import bacc, tile, mybir

    f32 = mybir.dt.float32
    f32r = mybir.dt.float32r
    i32 = mybir.dt.int32
    Alu = mybir.AluOpType
    Act = mybir.ActivationFunctionType

    nc = bacc.Bacc("TRN2", target_bir_lowering=False, debug=False, num_devices=NCORES)

    h_d = nc.dram_tensor("h", [BL, S, H], f32r, kind="ExternalInput")
    wid_d = nc.dram_tensor("wid", [BL, S], i32, kind="ExternalInput")
    tl_d = nc.dram_tensor("tl", [1, BL], i32, kind="ExternalInput")
    wt_d = nc.dram_tensor("wt", [H, E], f32r, kind="ExternalInput")  # W^T
    bias_d = nc.dram_tensor("bias", [1, E], f32r, kind="ExternalInput")
    ones_d = nc.dram_tensor("ones", [1, 128], f32r, kind="ExternalInput")
    out_d = nc.dram_tensor("out", [BL, T, E], f32, kind="ExternalOutput")

    with tile.TileContext(nc) as tc:
        with (
            tc.tile_pool(name="const", bufs=1) as cpool,
            tc.tile_pool(name="work", bufs=bufs) as wpool,
            tc.tile_pool(name="psum", bufs=1, space="PSUM") as ppool,
        ):
            # ---- one-time constants ----
            iota_i = cpool.tile([128, T], i32)
            nc.gpsimd.iota(iota_i[:], pattern=[[1, T]], base=0, channel_multiplier=0)
            iota_f = cpool.tile([128, T], f32)
            nc.vector.tensor_copy(iota_f[:], iota_i[:])
            ones_col = cpool.tile([128, 1], f32r)
            nc.sync.dma_start(ones_col[:], ones_d[0].rearrange("(p o) -> p o", o=1))
            tl_i = cpool.tile([1, BL], i32)
            nc.sync.dma_start(tl_i[:], tl_d[:])
            tl_f = cpool.tile([1, BL], f32)
            nc.vector.tensor_copy(tl_f[:], tl_i[:])
            wt_t = cpool.tile([128, KH, E], f32r)
            nc.sync.dma_start(wt_t[:], wt_d[:, :].rearrange("(k p) e -> p k e", p=128))
            bias_row = cpool.tile([1, E], f32r)
            nc.sync.dma_start(bias_row[:], bias_d[:])
            b_bc = cpool.tile([128, E], f32)
            nc.gpsimd.partition_broadcast(b_bc[:], bias_row[:].bitcast(f32))

            for i in range(BL * reps):
                i = i % BL
                # ---- load sentence: s is laid out as s = 4*p + k ----
                if widfirst:
                    wid_t = wpool.tile([128, KS], i32, tag="wid")
                    nc.sync.dma_start(wid_t[:], wid_d[i].rearrange("(p k) -> p k", k=KS))
                hs = wpool.tile([128, KS, H], f32r, tag="hs")
                h_src = h_d[i].rearrange("(p k) c -> p k c", k=KS)
                if dma3:
                    nc.sync.dma_start(hs[:, 0:2, :], h_src[:, 0:2, :])
                    nc.scalar.dma_start(hs[:, 2:3, :], h_src[:, 2:3, :])
                    nc.gpsimd.dma_start(hs[:, 3:4, :], h_src[:, 3:4, :])
                elif dma4:
                    nc.sync.dma_start(hs[:, 0:1, :], h_src[:, 0:1, :])
                    nc.scalar.dma_start(hs[:, 2:3, :], h_src[:, 2:3, :])
                    nc.sync.dma_start(hs[:, 1:2, :], h_src[:, 1:2, :])
                    nc.scalar.dma_start(hs[:, 3:4, :], h_src[:, 3:4, :])
                elif dma2:
                    nc.sync.dma_start(hs[:, 0:2, :], h_src[:, 0:2, :])
                    nc.scalar.dma_start(hs[:, 2:4, :], h_src[:, 2:4, :])
                else:
                    nc.sync.dma_start(hs[:], h_src)
                if not widfirst:
                    wid_t = wpool.tile([128, KS], i32, tag="wid")
                    nc.sync.dma_start(wid_t[:], wid_d[i].rearrange("(p k) -> p k", k=KS))
                wid_f = wpool.tile([128, KS], f32, tag="widf")
                nc.vector.tensor_copy(wid_f[:], wid_t[:])

                # ---- one-hot [s, t] ----
                oh = wpool.tile([128, KS, T], f32r, tag="oh")
                for k in range(KS):
                    nc.vector.tensor_scalar(
                        oh[:, k, :], iota_f[:], wid_f[:, k : k + 1], None, Alu.is_equal
                    )

                # ---- counts[1, t] = sum_s oh[s, t] ----
                counts_ps = ppool.tile([1, T], f32, tag="cnt")
                if "cnt" in ablated:
                    nc.vector.memset(counts_ps[:], 2.0)
                for k in range(KS if "cnt" not in ablated else 0):
                    nc.tensor.matmul(
                        counts_ps[:],
                        ones_col[:],
                        oh[:, k, :],
                        start=(k == 0),
                        stop=(k == KS - 1),
                    )

                # ---- scale[t] = (t < len) / max(counts, 1) ----
                cmax = wpool.tile([1, T], f32, tag="cmax")
                nc.vector.tensor_scalar(cmax[:], counts_ps[:], 1.0, None, Alu.max)
                crec = wpool.tile([1, T], f32, tag="crec")
                nc.vector.reciprocal(crec[:], cmax[:])
                mask = wpool.tile([1, T], f32, tag="mask")
                nc.vector.tensor_scalar(
                    mask[:], iota_f[0:1, :], tl_f[0:1, i : i + 1], None, Alu.is_lt
                )
                srow = wpool.tile([1, T], f32, tag="srow")
                nc.vector.tensor_tensor(srow[:], crec[:], mask[:], Alu.mult)
                # transpose scale_row -> per-partition scale columns [128, CT]
                scol = wpool.tile([128, CT], f32, tag="scol")
                scol_eng = nc.scalar if widfirst else nc.sync
                for c in range(CT):
                    scol_eng.dma_start(
                        scol[:, c : c + 1],
                        srow[0:1, c * 128 : (c + 1) * 128],
                    )

                # ---- matmul 1: pooled_sums^T [h, t] ----
                pt_ps = [
                    ppool.tile([128, 2 * T], f32, name=f"pt{j}", tag=f"pt{j}")
                    for j in range(3)
                ]
                for m in range(KH if "mm1" not in ablated else 0):
                    dst = pt_ps[m // 2][:, (m % 2) * T : (m % 2 + 1) * T]
                    for k in range(KS):
                        nc.tensor.matmul(
                            dst,
                            hs[:, k, m * 128 : (m + 1) * 128],
                            oh[:, k, :],
                            start=(k == 0),
                            stop=(k == KS - 1),
                        )

                # ---- move to SBUF (plain copies, split ACT/DVE) ----
                pts = wpool.tile([128, KH, T], f32r, tag="pts")
                if "mm1" in ablated:
                    for j in range(3):
                        nc.vector.memset(pt_ps[j][:], 0.5)
                for m in range(KH):
                    src_ap = pt_ps[m // 2][:, (m % 2) * T : (m % 2 + 1) * T]
                    if m % 2 == 0 and not ptsdve:
                        nc.scalar.copy(pts[:, m, :], src_ap)
                    else:
                        nc.vector.tensor_copy(pts[:, m, :], src_ap)

                # ---- matmul 2: out[t, e] = pooled @ W^T ----
                out_ps = [
                    ppool.tile([128, E], f32, name=f"o2{c}", tag=f"o2{c}", bufs=psum_bufs)
                    for c in range(CT)
                ]
                if "mm2" in ablated:
                    for c in range(CT):
                        nc.vector.memset(out_ps[c][:], 0.25)
                for c in range(CT if "mm2" not in ablated else 0):
                    for k in range(KH):
                        nc.tensor.matmul(
                            out_ps[c][:],
                            pts[:, k, c * 128 : (c + 1) * 128],
                            wt_t[:, k, :],
                            start=(k == 0),
                            stop=(k == KH - 1),
                        )

                # ---- scale (per-partition), add bias, PSUM -> SBUF -> DRAM ----
                outs = wpool.tile([128, CT, E], f32, tag="outs")
                for c in range(CT):
                    nc.scalar.activation(
                        outs[:, c, :],
                        out_ps[c][:],
                        Act.Copy,
                        scale=scol[:, c : c + 1],
                    )
                    nc.vector.tensor_tensor(outs[:, c, :], outs[:, c, :], b_bc[:], Alu.add)
                if outmerge:
                    eng = nc.scalar if i % 2 else nc.sync
                    eng.dma_start(
                        out_d[i].rearrange("(c p) e -> p c e", c=CT), outs[:, :, :]
                    )
                else:
                    for c in range(CT):
                        eng = nc.scalar if (dma2 and c == 1) else nc.sync
                        eng.dma_start(out_d[i, c * 128 : (c + 1) * 128, :], outs[:, c, :])

    nc.compile()
    return nc


def _get_nc(reps: int = 1, **opts):
    key = f"nc{reps}|{sorted(opts.items())}"
    if key not in _cache:
        _cache[key] = _build(reps, **opts)
    return _cache[key]


def _in_maps(hidden_states, word_ids, token_lengths, W, b):
    wt = np.ascontiguousarray(W.T.astype(np.float32, copy=False))
    bias = np.ascontiguousarray(b.astype(np.float32, copy=False)).reshape(1, E)
    maps = []
    for c in range(NCORES):
        sl = slice(c * BL, (c + 1) * BL)
        maps.append(
            {
                "h": np.ascontiguousarray(hidden_states[sl]).astype(np.float32, copy=False),
                "wid": np.ascontiguousarray(word_ids[sl]).astype(np.int32, copy=False),
                "tl": np.ascontiguousarray(token_lengths[sl]).astype(np.int32, copy=False).reshape(1, BL),
                "wt": wt,
                "bias": bias,
                "ones": np.ones((1, 128), np.float32),
            }
        )
    return maps


def kernel(hidden_states, word_ids, token_lengths, W, b):
    from concourse import bass_utils

    nc = _get_nc()
    maps = _in_maps(hidden_states, word_ids, token_lengths, W, b)
    res = bass_utils.run_bass_kernel_spmd(nc, maps, core_ids=list(range(NCORES)))
    out = np.concatenate([res.results[c]["out"] for c in range(NCORES)], axis=0)
    return out

